# revision 17
# baseline (speedup 1.0000x reference)
"""Chamfer distance 2D (B=8, N=M=8192) Trainium2 Bass kernel.

Strategy
--------
Data-parallel over batch: core b handles batch b (its own 8192x8192 chamfer).

Per core, two symmetric passes (p1->p2 and p2->p1). Each pass computes the
full squared-distance matrix via a K=18 bf16 matmul using split arithmetic:

  d[n, m] = |p1[n]|^2 + |p2[m]|^2 - 2 p1[n].p2[m]

Each fp32 scalar is split into 3 bf16 components (hi/mid/lo, ~24 mantissa
bits recovered); products keep the 6 dominant component pairs per
coordinate, plus 3 rows for each squared-norm term against ones. PSUM
accumulates in fp32, so the matmul emits full squared distances directly
(error ~3e-7 * |p|^2, below the fp32 reference's own rounding noise).

Matmuls are packed 4x with tile_position row groups (K=18 <= 32): row group
r computes chunk 4g+r of the same query tile, filling one [128, 2048] PSUM
tile (4 banks) per group. LDWEIGHTS overlaps across row groups.

Row-min reduction is the bottleneck (only DVE can reduce, only DVE/ACT can
read PSUM, 1 elem/cycle/lane each). Tiles are routed two ways to keep both
engines busy:
  - direct: DVE tensor_reduce(min) straight from PSUM
  - offload: ScalarE copies PSUM -> SBUF bf16, then DVE combines leaves with
    2x-mode bf16 tensor_tensor(min) and one final 1x reduce
Per query tile (4 PSUM tiles), either 3 or 4 tiles take the offload path;
the mix is tuned so DVE and ScalarE finish together.

Output: per-core [128, 2] partition-wise sums of row minima (pass A, B).
Host sums partitions, divides by 8192, adds over batches.
"""
import os
import numpy as np
import ml_dtypes

import concourse.bass as bass
import concourse.tile as tile
from concourse import bacc, mybir, bass_utils
from concourse.bass import ts

f32 = mybir.dt.float32
bf16 = mybir.dt.bfloat16
BIG = 3.0e38

B, N, M = 8, 8192, 8192
NT = 64        # query tiles of 128 per pass
NG = 4         # chunk groups per query tile (each = 4 chunks of 512 = [128, 2048])
K = 18
# per-query-tile offload count (number of the 4 groups routed via ACT).
# t=3 (1 direct PSUM reduce + 3 ACT leaves) for most tiles, t=4 for 1 in 16:
# balances DVE and ScalarE with the batched collapse chain in place.
OFFLOAD_T = [4 if (i % 16) == 0 else 3 for i in range(NT)]

_CACHE = {}


def _build_program():
    nc = bacc.Bacc("TRN2", target_bir_lowering=False, debug=False)
    qa_a = nc.dram_tensor("qa_a", [K, N], bf16, kind="ExternalInput")
    db_a = nc.dram_tensor("db_a", [K, M], bf16, kind="ExternalInput")
    qa_b = nc.dram_tensor("qa_b", [K, M], bf16, kind="ExternalInput")
    db_b = nc.dram_tensor("db_b", [K, N], bf16, kind="ExternalInput")
    out = nc.dram_tensor("out", [128, 2], f32, kind="ExternalOutput")

    with tile.TileContext(nc) as tc:
        with (
            tc.tile_pool(name="inp", bufs=1) as inp,
            tc.tile_pool(name="psum", bufs=2, space="PSUM") as psum,
            tc.tile_pool(name="conv", bufs=10) as conv,
            tc.tile_pool(name="tree", bufs=3) as tree,
            tc.tile_pool(name="big", bufs=2) as big,
            tc.tile_pool(name="accs", bufs=1) as accs,
        ):
            reps = {}
            for name, dram in (("qa_a", qa_a), ("db_a", db_a),
                               ("qa_b", qa_b), ("db_b", db_b)):
                t = inp.tile([128, 8192], bf16, tag=f"rep_{name}")
                for r in range(4):
                    nc.sync.dma_start(out=t[32 * r:32 * r + K, :], in_=dram.ap())
                reps[name] = t

            out_sb = accs.tile([128, 2], f32)

            for ipass, (qa_t, db_t) in enumerate(
                ((reps["qa_a"], reps["db_a"]), (reps["qa_b"], reps["db_b"]))
            ):
                minacc = accs.tile([128, NT, NG], f32, tag=f"minacc{ipass}")
                nc.vector.memset(minacc[:], BIG)

                # Software-pipelined emission: each query tile's tree TTs are
                # deferred by TWO tiles so the DVE queue (in-order) only sees
                # ops whose ACT-converted inputs are already resident.
                pending = []    # [(nt, leaves), ...] awaiting tree emission
                big4 = None     # current batch's root buffer
                pending_chain = None  # (nt0, big4) awaiting collapse chain

                def emit_tree(nt, leaves):
                    nonlocal big4
                    j = nt % 4
                    if j == 0:
                        big4 = big.tile([128, 4, 2048], bf16, tag="big4")
                    while len(leaves) > 2:
                        a = leaves.pop(0)
                        b = leaves.pop(0)
                        u = tree.tile([128, 2048], bf16, tag="tr")
                        nc.vector.tensor_tensor(
                            out=u[:], in0=a[:], in1=b[:], op=mybir.AluOpType.min
                        )
                        leaves.append(u)
                    nc.vector.tensor_tensor(
                        out=big4[:, j, :], in0=leaves[0][:], in1=leaves[1][:],
                        op=mybir.AluOpType.min,
                    )
                    return (nt - 3, big4) if j == 3 else None

                def emit_chain(nt0, roots):
                    c1 = big.tile([128, 4, 1024], bf16, tag="c1")
                    nc.vector.tensor_tensor(
                        out=c1[:], in0=roots[:, :, 0:1024], in1=roots[:, :, 1024:2048],
                        op=mybir.AluOpType.min,
                    )
                    c2 = big.tile([128, 4, 512], bf16, tag="c2")
                    nc.vector.tensor_tensor(
                        out=c2[:], in0=c1[:, :, 0:512], in1=c1[:, :, 512:1024],
                        op=mybir.AluOpType.min,
                    )
                    nc.vector.tensor_reduce(
                        out=minacc[:, nt0:nt0 + 4, NG - 1:NG], in_=c2[:],
                        axis=mybir.AxisListType.X, op=mybir.AluOpType.min,
                    )

                for nt in range(NT):
                    # deferred emission (two tiles behind) FIRST so the DVE
                    # queue leads with ready work
                    if len(pending) >= 2:
                        done = emit_tree(*pending.pop(0))
                        if done is not None:
                            pending_chain = done
                    if pending_chain is not None and (nt % 4) == 2:
                        emit_chain(*pending_chain)
                        pending_chain = None
                    t_off = OFFLOAD_T[nt]
                    leaves = []
                    for g in range(NG):
                        pt = psum.tile([128, 2048], f32, tag="ps")
                        for r in range(4):
                            c = 4 * g + r
                            nc.tensor.matmul(
                                out=pt[:, ts(r, 512)],
                                lhsT=qa_t[32 * r:32 * r + K, ts(nt, 128)],
                                rhs=db_t[32 * r:32 * r + K, ts(c, 512)],
                                start=True, stop=True,
                                tile_position=(32 * r, 0),
                            )
                        if g >= t_off:
                            # direct group last: DVE reduce straight from PSUM
                            nc.vector.tensor_reduce(
                                out=minacc[:, nt, 0:1], in_=pt[:],
                                axis=mybir.AxisListType.X, op=mybir.AluOpType.min,
                            )
                        else:
                            leaf = conv.tile([128, 2048], bf16, tag="leaf")
                            nc.scalar.copy(out=leaf[:], in_=pt[:])
                            leaves.append(leaf)
                    pending.append((nt, leaves))

                while pending:
                    done = emit_tree(*pending.pop(0))
                    if pending_chain is not None:
                        emit_chain(*pending_chain)
                        pending_chain = None
                    if done is not None:
                        pending_chain = done
                if pending_chain is not None:
                    emit_chain(*pending_chain)
                    pending_chain = None

                # per-tile min over the NG slots, then sum over tiles
                tile_mins = accs.tile([128, NT], f32, tag=f"tm{ipass}")
                nc.vector.tensor_reduce(
                    out=tile_mins[:], in_=minacc[:],
                    axis=mybir.AxisListType.X, op=mybir.AluOpType.min,
                )
                nc.vector.tensor_reduce(
                    out=out_sb[:, ipass:ipass + 1], in_=tile_mins[:],
                    axis=mybir.AxisListType.X, op=mybir.AluOpType.add,
                )

            nc.sync.dma_start(out=out.ap(), in_=out_sb[:])

    nc.compile()
    return nc


def _split3(x):
    h = x.astype(ml_dtypes.bfloat16).astype(np.float32)
    m = (x - h).astype(ml_dtypes.bfloat16).astype(np.float32)
    l = (x - h - m).astype(ml_dtypes.bfloat16).astype(np.float32)
    return h, m, l


def _make_aug(pq, pdb):
    """Query-side aug [18, Nq] and db-side aug [18, Nd] (bf16)."""
    xqh, xqm, xql = _split3(np.ascontiguousarray(pq[:, 0]))
    yqh, yqm, yql = _split3(np.ascontiguousarray(pq[:, 1]))
    xdh, xdm, xdl = _split3(np.ascontiguousarray(pdb[:, 0]))
    ydh, ydm, ydl = _split3(np.ascontiguousarray(pdb[:, 1]))
    sq = (pq.astype(np.float64) ** 2).sum(1).astype(np.float32)
    sd = (pdb.astype(np.float64) ** 2).sum(1).astype(np.float32)
    sqh, sqm, sql = _split3(sq)
    sdh, sdm, sdl = _split3(sd)
    oq = np.ones(len(pq), np.float32)
    od = np.ones(len(pdb), np.float32)
    # pair layout: 6 x-terms, 6 y-terms, 3 q-norm rows, 3 d-norm rows
    q_aug = np.stack([
        xqh, xqh, xqm, xqh, xql, xqm,
        yqh, yqh, yqm, yqh, yql, yqm,
        sqh, sqm, sql,
        oq, oq, oq,
    ])
    d_aug = np.stack([
        -2 * xdh, -2 * xdm, -2 * xdh, -2 * xdl, -2 * xdh, -2 * xdm,
        -2 * ydh, -2 * ydm, -2 * ydh, -2 * ydl, -2 * ydh, -2 * ydm,
        od, od, od,
        sdh, sdm, sdl,
    ])
    return (q_aug.astype(ml_dtypes.bfloat16), d_aug.astype(ml_dtypes.bfloat16))


def kernel(points1, points2):
    points1 = np.asarray(points1, dtype=np.float32)
    points2 = np.asarray(points2, dtype=np.float32)
    assert points1.shape == (B, N, 2) and points2.shape == (B, M, 2)

    if "nc" not in _CACHE:
        _CACHE["nc"] = _build_program()
    nc = _CACHE["nc"]

    in_maps = []
    for b in range(B):
        qa_a, db_a = _make_aug(points1[b], points2[b])
        qa_b, db_b = _make_aug(points2[b], points1[b])
        in_maps.append({"qa_a": qa_a, "db_a": db_a, "qa_b": qa_b, "db_b": db_b})

    trace = bool(int(os.environ.get("CHAMFER_TRACE", "0")))
    kw = {}
    if trace:
        import tempfile
        bass_utils.upload_artifacts = lambda d: f"local:{d}"
        kw = dict(trace=True, tmpdir=tempfile.mkdtemp())
    res = bass_utils.run_bass_kernel_spmd(nc, in_maps, core_ids=list(range(B)), **kw)
    if trace:
        _CACHE["last_exec_time_ns"] = res.exec_time_ns
        _CACHE["last_results"] = res

    total = np.float32(0.0)
    for b in range(B):
        sums = res.results[b]["out"]  # [128, 2]
        cost = sums[:, 0].sum(dtype=np.float32) / np.float32(N) \
            + sums[:, 1].sum(dtype=np.float32) / np.float32(M)
        total = np.float32(total + cost)
    return np.array(total, dtype=np.float32)


# revision 18
# speedup vs baseline: 1.0625x; 1.0625x over previous
"""Chamfer distance 2D (B=8, N=M=8192) Trainium2 Bass kernel.

Strategy
--------
Data-parallel over batch: core b handles batch b (its own 8192x8192 chamfer).

Per core, two symmetric passes (p1->p2 and p2->p1). Each pass computes the
full squared-distance matrix via a K=18 bf16 matmul using split arithmetic:

  d[n, m] = |p1[n]|^2 + |p2[m]|^2 - 2 p1[n].p2[m]

Each fp32 scalar is split into 3 bf16 components (hi/mid/lo, ~24 mantissa
bits recovered); products keep the 6 dominant component pairs per
coordinate, plus 3 rows for each squared-norm term against ones. PSUM
accumulates in fp32, so the matmul emits full squared distances directly
(error ~3e-7 * |p|^2, below the fp32 reference's own rounding noise).

Matmuls are packed 4x with tile_position row groups (K=18 <= 32): row group
r computes chunk 4g+r of the same query tile, filling one [128, 2048] PSUM
tile (4 banks) per group. LDWEIGHTS overlaps across row groups.

Row-min reduction is the bottleneck (only DVE can reduce, only DVE/ACT can
read PSUM, 1 elem/cycle/lane each). Tiles are routed two ways to keep both
engines busy:
  - direct: DVE tensor_reduce(min) straight from PSUM
  - offload: ScalarE copies PSUM -> SBUF bf16, then DVE combines leaves with
    2x-mode bf16 tensor_tensor(min) and one final 1x reduce
Per query tile (4 PSUM tiles), either 3 or 4 tiles take the offload path;
the mix is tuned so DVE and ScalarE finish together.

Output: per-core [128, 2] partition-wise sums of row minima (pass A, B).
Host sums partitions, divides by 8192, adds over batches.
"""
import os
import numpy as np
import ml_dtypes

import concourse.bass as bass
import concourse.tile as tile
from concourse import bacc, mybir, bass_utils
from concourse.bass import ts

f32 = mybir.dt.float32
bf16 = mybir.dt.bfloat16
BIG = 3.0e38

B, N, M = 8, 8192, 8192
NT = 64        # query tiles of 128 per pass
NG = 4         # chunk groups per query tile (each = 4 chunks of 512 = [128, 2048])
K = 18
# per-query-tile offload count (number of the 4 groups routed via ACT).
# t=3 (1 direct PSUM reduce + 3 ACT leaves) for most tiles, t=4 for 1 in 16:
# balances DVE and ScalarE with the batched collapse chain in place.
OFFLOAD_T = [4 if (i % 16) == 0 else 3 for i in range(NT)]

_CACHE = {}


def _build_program():
    nc = bacc.Bacc("TRN2", target_bir_lowering=False, debug=False)
    qa_a = nc.dram_tensor("qa_a", [K, N], bf16, kind="ExternalInput")
    db_a = nc.dram_tensor("db_a", [K, M], bf16, kind="ExternalInput")
    qa_b = nc.dram_tensor("qa_b", [K, M], bf16, kind="ExternalInput")
    db_b = nc.dram_tensor("db_b", [K, N], bf16, kind="ExternalInput")
    out = nc.dram_tensor("out", [128, 2], f32, kind="ExternalOutput")

    with tile.TileContext(nc) as tc:
        with (
            tc.tile_pool(name="inp", bufs=1) as inp,
            tc.tile_pool(name="psum", bufs=2, space="PSUM") as psum,
            tc.tile_pool(name="conv", bufs=10) as conv,
            tc.tile_pool(name="tree", bufs=3) as tree,
            tc.tile_pool(name="big", bufs=2) as big,
            tc.tile_pool(name="accs", bufs=1) as accs,
        ):
            reps = {}
            for name, dram in (("qa_a", qa_a), ("db_a", db_a),
                               ("qa_b", qa_b), ("db_b", db_b)):
                t = inp.tile([128, 8192], bf16, tag=f"rep_{name}")
                for r in range(4):
                    nc.sync.dma_start(out=t[32 * r:32 * r + K, :], in_=dram.ap())
                reps[name] = t

            out_sb = accs.tile([128, 2], f32)

            for ipass, (qa_t, db_t) in enumerate(
                ((reps["qa_a"], reps["db_a"]), (reps["qa_b"], reps["db_b"]))
            ):
                minacc = accs.tile([128, NT, NG], f32, tag=f"minacc{ipass}")
                nc.vector.memset(minacc[:], BIG)

                # Software-pipelined emission: each query tile's tree TTs are
                # deferred by TWO tiles so the DVE queue (in-order) only sees
                # ops whose ACT-converted inputs are already resident.
                pending = []    # [(nt, leaves), ...] awaiting tree emission
                big4 = None     # current batch's root buffer
                pending_chain = None  # (nt0, big4) awaiting collapse chain

                def emit_tree(nt, leaves):
                    nonlocal big4
                    j = nt % 4
                    if j == 0:
                        big4 = big.tile([128, 4, 2048], bf16, tag="big4")
                    while len(leaves) > 2:
                        a = leaves.pop(0)
                        b = leaves.pop(0)
                        u = tree.tile([128, 2048], bf16, tag="tr")
                        nc.vector.tensor_tensor(
                            out=u[:], in0=a[:], in1=b[:], op=mybir.AluOpType.min
                        )
                        leaves.append(u)
                    nc.vector.tensor_tensor(
                        out=big4[:, j, :], in0=leaves[0][:], in1=leaves[1][:],
                        op=mybir.AluOpType.min,
                    )
                    return (nt - 3, big4) if j == 3 else None

                def emit_chain(nt0, roots):
                    c1 = big.tile([128, 4, 1024], bf16, tag="c1")
                    nc.vector.tensor_tensor(
                        out=c1[:], in0=roots[:, :, 0:1024], in1=roots[:, :, 1024:2048],
                        op=mybir.AluOpType.min,
                    )
                    c2 = big.tile([128, 4, 512], bf16, tag="c2")
                    nc.vector.tensor_tensor(
                        out=c2[:], in0=c1[:, :, 0:512], in1=c1[:, :, 512:1024],
                        op=mybir.AluOpType.min,
                    )
                    nc.vector.tensor_reduce(
                        out=minacc[:, nt0:nt0 + 4, NG - 1:NG], in_=c2[:],
                        axis=mybir.AxisListType.X, op=mybir.AluOpType.min,
                    )

                for nt in range(NT):
                    # deferred emission (two tiles behind) FIRST so the DVE
                    # queue leads with ready work
                    if len(pending) >= 2:
                        done = emit_tree(*pending.pop(0))
                        if done is not None:
                            pending_chain = done
                    if pending_chain is not None and (nt % 4) == 2:
                        emit_chain(*pending_chain)
                        pending_chain = None
                    t_off = OFFLOAD_T[nt]
                    leaves = []
                    for g in range(NG):
                        if g < t_off:
                            # conv group: two 2-bank tiles so ACT's fills
                            # ping-pong and its convs run back-to-back
                            sa = psum.tile([128, 1024], f32, tag="ps_s", bufs=2)
                            sb = psum.tile([128, 1024], f32, tag="ps_s", bufs=2)
                            for r in range(4):
                                c = 4 * g + r
                                dst = (sa, sb)[r // 2][:, ts(r % 2, 512)]
                                nc.tensor.matmul(
                                    out=dst,
                                    lhsT=qa_t[32 * r:32 * r + K, ts(nt, 128)],
                                    rhs=db_t[32 * r:32 * r + K, ts(c, 512)],
                                    start=True, stop=True,
                                    tile_position=(32 * r, 0),
                                )
                            leaf = conv.tile([128, 2048], bf16, tag="leaf")
                            nc.scalar.copy(out=leaf[:, 0:1024], in_=sa[:])
                            nc.scalar.copy(out=leaf[:, 1024:2048], in_=sb[:])
                            leaves.append(leaf)
                        else:
                            # direct group: own 4-bank tile, DVE reduces PSUM
                            pt = psum.tile([128, 2048], f32, tag="ps_big", bufs=1)
                            for r in range(4):
                                c = 4 * g + r
                                nc.tensor.matmul(
                                    out=pt[:, ts(r, 512)],
                                    lhsT=qa_t[32 * r:32 * r + K, ts(nt, 128)],
                                    rhs=db_t[32 * r:32 * r + K, ts(c, 512)],
                                    start=True, stop=True,
                                    tile_position=(32 * r, 0),
                                )
                            nc.vector.tensor_reduce(
                                out=minacc[:, nt, 0:1], in_=pt[:],
                                axis=mybir.AxisListType.X, op=mybir.AluOpType.min,
                            )
                    pending.append((nt, leaves))

                while pending:
                    done = emit_tree(*pending.pop(0))
                    if pending_chain is not None:
                        emit_chain(*pending_chain)
                        pending_chain = None
                    if done is not None:
                        pending_chain = done
                if pending_chain is not None:
                    emit_chain(*pending_chain)
                    pending_chain = None

                # per-tile min over the NG slots, then sum over tiles
                tile_mins = accs.tile([128, NT], f32, tag=f"tm{ipass}")
                nc.vector.tensor_reduce(
                    out=tile_mins[:], in_=minacc[:],
                    axis=mybir.AxisListType.X, op=mybir.AluOpType.min,
                )
                nc.vector.tensor_reduce(
                    out=out_sb[:, ipass:ipass + 1], in_=tile_mins[:],
                    axis=mybir.AxisListType.X, op=mybir.AluOpType.add,
                )

            nc.sync.dma_start(out=out.ap(), in_=out_sb[:])

    nc.compile()
    return nc


def _split3(x):
    h = x.astype(ml_dtypes.bfloat16).astype(np.float32)
    m = (x - h).astype(ml_dtypes.bfloat16).astype(np.float32)
    l = (x - h - m).astype(ml_dtypes.bfloat16).astype(np.float32)
    return h, m, l


def _make_aug(pq, pdb):
    """Query-side aug [18, Nq] and db-side aug [18, Nd] (bf16)."""
    xqh, xqm, xql = _split3(np.ascontiguousarray(pq[:, 0]))
    yqh, yqm, yql = _split3(np.ascontiguousarray(pq[:, 1]))
    xdh, xdm, xdl = _split3(np.ascontiguousarray(pdb[:, 0]))
    ydh, ydm, ydl = _split3(np.ascontiguousarray(pdb[:, 1]))
    sq = (pq.astype(np.float64) ** 2).sum(1).astype(np.float32)
    sd = (pdb.astype(np.float64) ** 2).sum(1).astype(np.float32)
    sqh, sqm, sql = _split3(sq)
    sdh, sdm, sdl = _split3(sd)
    oq = np.ones(len(pq), np.float32)
    od = np.ones(len(pdb), np.float32)
    # pair layout: 6 x-terms, 6 y-terms, 3 q-norm rows, 3 d-norm rows
    q_aug = np.stack([
        xqh, xqh, xqm, xqh, xql, xqm,
        yqh, yqh, yqm, yqh, yql, yqm,
        sqh, sqm, sql,
        oq, oq, oq,
    ])
    d_aug = np.stack([
        -2 * xdh, -2 * xdm, -2 * xdh, -2 * xdl, -2 * xdh, -2 * xdm,
        -2 * ydh, -2 * ydm, -2 * ydh, -2 * ydl, -2 * ydh, -2 * ydm,
        od, od, od,
        sdh, sdm, sdl,
    ])
    return (q_aug.astype(ml_dtypes.bfloat16), d_aug.astype(ml_dtypes.bfloat16))


def kernel(points1, points2):
    points1 = np.asarray(points1, dtype=np.float32)
    points2 = np.asarray(points2, dtype=np.float32)
    assert points1.shape == (B, N, 2) and points2.shape == (B, M, 2)

    if "nc" not in _CACHE:
        _CACHE["nc"] = _build_program()
    nc = _CACHE["nc"]

    in_maps = []
    for b in range(B):
        qa_a, db_a = _make_aug(points1[b], points2[b])
        qa_b, db_b = _make_aug(points2[b], points1[b])
        in_maps.append({"qa_a": qa_a, "db_a": db_a, "qa_b": qa_b, "db_b": db_b})

    trace = bool(int(os.environ.get("CHAMFER_TRACE", "0")))
    kw = {}
    if trace:
        import tempfile
        bass_utils.upload_artifacts = lambda d: f"local:{d}"
        kw = dict(trace=True, tmpdir=tempfile.mkdtemp())
    res = bass_utils.run_bass_kernel_spmd(nc, in_maps, core_ids=list(range(B)), **kw)
    if trace:
        _CACHE["last_exec_time_ns"] = res.exec_time_ns
        _CACHE["last_results"] = res

    total = np.float32(0.0)
    for b in range(B):
        sums = res.results[b]["out"]  # [128, 2]
        cost = sums[:, 0].sum(dtype=np.float32) / np.float32(N) \
            + sums[:, 1].sum(dtype=np.float32) / np.float32(M)
        total = np.float32(total + cost)
    return np.array(total, dtype=np.float32)


# revision 20
# speedup vs baseline: 1.0827x; 1.0190x over previous
"""Chamfer distance 2D (B=8, N=M=8192) Trainium2 Bass kernel.

Strategy
--------
Data-parallel over batch: core b handles batch b (its own 8192x8192 chamfer).

Per core, two symmetric passes (p1->p2 and p2->p1). Each pass computes the
full squared-distance matrix via a K=18 bf16 matmul using split arithmetic:

  d[n, m] = |p1[n]|^2 + |p2[m]|^2 - 2 p1[n].p2[m]

Each fp32 scalar is split into 3 bf16 components (hi/mid/lo, ~24 mantissa
bits recovered); products keep the 6 dominant component pairs per
coordinate, plus 3 rows for each squared-norm term against ones. PSUM
accumulates in fp32, so the matmul emits full squared distances directly
(error ~3e-7 * |p|^2, below the fp32 reference's own rounding noise).

Matmuls are packed 4x with tile_position row groups (K=18 <= 32): row group
r computes chunk 4g+r of the same query tile, filling one [128, 2048] PSUM
tile (4 banks) per group. LDWEIGHTS overlaps across row groups.

Row-min reduction is the bottleneck (only DVE can reduce, only DVE/ACT can
read PSUM, 1 elem/cycle/lane each). Tiles are routed two ways to keep both
engines busy:
  - direct: DVE tensor_reduce(min) straight from PSUM
  - offload: ScalarE copies PSUM -> SBUF bf16, then DVE combines leaves with
    2x-mode bf16 tensor_tensor(min) and one final 1x reduce
Per query tile (4 PSUM tiles), either 3 or 4 tiles take the offload path;
the mix is tuned so DVE and ScalarE finish together.

Output: per-core [128, 2] partition-wise sums of row minima (pass A, B).
Host sums partitions, divides by 8192, adds over batches.
"""
import os
import numpy as np
import ml_dtypes

import concourse.bass as bass
import concourse.tile as tile
from concourse import bacc, mybir, bass_utils
from concourse.bass import ts

f32 = mybir.dt.float32
bf16 = mybir.dt.bfloat16
BIG = 3.0e38

B, N, M = 8, 8192, 8192
NT = 64        # query tiles of 128 per pass
NG = 4         # chunk groups per query tile (each = 4 chunks of 512 = [128, 2048])
K = 18
# per-query-tile offload count (number of the 4 groups routed via ACT).
# t=3 (1 direct PSUM reduce + 3 ACT leaves) everywhere: with the hybrid PSUM
# layout this balances DVE (~5.9us/tile) and ScalarE (~6.0us/tile).
OFFLOAD_T = [3 for i in range(NT)]

_CACHE = {}


def _build_program():
    nc = bacc.Bacc("TRN2", target_bir_lowering=False, debug=False)
    qa_a = nc.dram_tensor("qa_a", [K, N], bf16, kind="ExternalInput")
    db_a = nc.dram_tensor("db_a", [K, M], bf16, kind="ExternalInput")
    qa_b = nc.dram_tensor("qa_b", [K, M], bf16, kind="ExternalInput")
    db_b = nc.dram_tensor("db_b", [K, N], bf16, kind="ExternalInput")
    out = nc.dram_tensor("out", [128, 2], f32, kind="ExternalOutput")

    with tile.TileContext(nc) as tc:
        with (
            tc.tile_pool(name="inp", bufs=1) as inp,
            tc.tile_pool(name="psum", bufs=2, space="PSUM") as psum,
            tc.tile_pool(name="conv", bufs=10) as conv,
            tc.tile_pool(name="tree", bufs=3) as tree,
            tc.tile_pool(name="big", bufs=2) as big,
            tc.tile_pool(name="accs", bufs=1) as accs,
        ):
            reps = {}
            for name, dram in (("qa_a", qa_a), ("db_a", db_a),
                               ("qa_b", qa_b), ("db_b", db_b)):
                t = inp.tile([128, 8192], bf16, tag=f"rep_{name}")
                for r in range(4):
                    nc.sync.dma_start(out=t[32 * r:32 * r + K, :], in_=dram.ap())
                reps[name] = t

            out_sb = accs.tile([128, 2], f32)

            for ipass, (qa_t, db_t) in enumerate(
                ((reps["qa_a"], reps["db_a"]), (reps["qa_b"], reps["db_b"]))
            ):
                minacc = accs.tile([128, NT, NG], f32, tag=f"minacc{ipass}")
                nc.vector.memset(minacc[:], BIG)

                # Software-pipelined emission: each query tile's tree TTs are
                # deferred by TWO tiles so the DVE queue (in-order) only sees
                # ops whose ACT-converted inputs are already resident.
                pending = []    # [(nt, leaves), ...] awaiting tree emission
                big4 = None     # current batch's root buffer
                pending_chain = None  # (nt0, big4) awaiting collapse chain

                def emit_tree(nt, leaves):
                    nonlocal big4
                    j = nt % 4
                    if j == 0:
                        big4 = big.tile([128, 4, 2048], bf16, tag="big4")
                    while len(leaves) > 2:
                        a = leaves.pop(0)
                        b = leaves.pop(0)
                        u = tree.tile([128, 2048], bf16, tag="tr")
                        nc.vector.tensor_tensor(
                            out=u[:], in0=a[:], in1=b[:], op=mybir.AluOpType.min
                        )
                        leaves.append(u)
                    nc.vector.tensor_tensor(
                        out=big4[:, j, :], in0=leaves[0][:], in1=leaves[1][:],
                        op=mybir.AluOpType.min,
                    )
                    return (nt - 3, big4) if j == 3 else None

                def emit_chain(nt0, roots):
                    c1 = big.tile([128, 4, 1024], bf16, tag="c1")
                    nc.vector.tensor_tensor(
                        out=c1[:], in0=roots[:, :, 0:1024], in1=roots[:, :, 1024:2048],
                        op=mybir.AluOpType.min,
                    )
                    c2 = big.tile([128, 4, 512], bf16, tag="c2")
                    nc.vector.tensor_tensor(
                        out=c2[:], in0=c1[:, :, 0:512], in1=c1[:, :, 512:1024],
                        op=mybir.AluOpType.min,
                    )
                    c3 = big.tile([128, 4, 256], bf16, tag="c3")
                    nc.vector.tensor_tensor(
                        out=c3[:], in0=c2[:, :, 0:256], in1=c2[:, :, 256:512],
                        op=mybir.AluOpType.min,
                    )
                    nc.vector.tensor_reduce(
                        out=minacc[:, nt0:nt0 + 4, NG - 1:NG], in_=c3[:],
                        axis=mybir.AxisListType.X, op=mybir.AluOpType.min,
                    )

                for nt in range(NT):
                    # deferred emission (two tiles behind) FIRST so the DVE
                    # queue leads with ready work
                    if len(pending) >= 2:
                        done = emit_tree(*pending.pop(0))
                        if done is not None:
                            pending_chain = done
                    if pending_chain is not None and (nt % 4) == 2:
                        emit_chain(*pending_chain)
                        pending_chain = None
                    t_off = OFFLOAD_T[nt]
                    leaves = []
                    for g in range(NG):
                        if g < t_off:
                            # conv group: two 2-bank tiles so ACT's fills
                            # ping-pong and its convs run back-to-back
                            sa = psum.tile([128, 1024], f32, tag="ps_s", bufs=2)
                            sb = psum.tile([128, 1024], f32, tag="ps_s", bufs=2)
                            for r in range(4):
                                c = 4 * g + r
                                dst = (sa, sb)[r // 2][:, ts(r % 2, 512)]
                                nc.tensor.matmul(
                                    out=dst,
                                    lhsT=qa_t[32 * r:32 * r + K, ts(nt, 128)],
                                    rhs=db_t[32 * r:32 * r + K, ts(c, 512)],
                                    start=True, stop=True,
                                    tile_position=(32 * r, 0),
                                )
                            leaf = conv.tile([128, 2048], bf16, tag="leaf")
                            nc.scalar.copy(out=leaf[:, 0:1024], in_=sa[:])
                            nc.scalar.copy(out=leaf[:, 1024:2048], in_=sb[:])
                            leaves.append(leaf)
                        else:
                            # direct group: own 4-bank tile, DVE reduces PSUM
                            pt = psum.tile([128, 2048], f32, tag="ps_big", bufs=1)
                            for r in range(4):
                                c = 4 * g + r
                                nc.tensor.matmul(
                                    out=pt[:, ts(r, 512)],
                                    lhsT=qa_t[32 * r:32 * r + K, ts(nt, 128)],
                                    rhs=db_t[32 * r:32 * r + K, ts(c, 512)],
                                    start=True, stop=True,
                                    tile_position=(32 * r, 0),
                                )
                            nc.vector.tensor_reduce(
                                out=minacc[:, nt, 0:1], in_=pt[:],
                                axis=mybir.AxisListType.X, op=mybir.AluOpType.min,
                            )
                    pending.append((nt, leaves))

                while pending:
                    done = emit_tree(*pending.pop(0))
                    if pending_chain is not None:
                        emit_chain(*pending_chain)
                        pending_chain = None
                    if done is not None:
                        pending_chain = done
                if pending_chain is not None:
                    emit_chain(*pending_chain)
                    pending_chain = None

                # per-tile min over the NG slots, then sum over tiles
                tile_mins = accs.tile([128, NT], f32, tag=f"tm{ipass}")
                nc.vector.tensor_reduce(
                    out=tile_mins[:], in_=minacc[:],
                    axis=mybir.AxisListType.X, op=mybir.AluOpType.min,
                )
                nc.vector.tensor_reduce(
                    out=out_sb[:, ipass:ipass + 1], in_=tile_mins[:],
                    axis=mybir.AxisListType.X, op=mybir.AluOpType.add,
                )

            nc.sync.dma_start(out=out.ap(), in_=out_sb[:])

    nc.compile()
    return nc


def _split3(x):
    h = x.astype(ml_dtypes.bfloat16).astype(np.float32)
    m = (x - h).astype(ml_dtypes.bfloat16).astype(np.float32)
    l = (x - h - m).astype(ml_dtypes.bfloat16).astype(np.float32)
    return h, m, l


def _make_aug(pq, pdb):
    """Query-side aug [18, Nq] and db-side aug [18, Nd] (bf16)."""
    xqh, xqm, xql = _split3(np.ascontiguousarray(pq[:, 0]))
    yqh, yqm, yql = _split3(np.ascontiguousarray(pq[:, 1]))
    xdh, xdm, xdl = _split3(np.ascontiguousarray(pdb[:, 0]))
    ydh, ydm, ydl = _split3(np.ascontiguousarray(pdb[:, 1]))
    sq = (pq.astype(np.float64) ** 2).sum(1).astype(np.float32)
    sd = (pdb.astype(np.float64) ** 2).sum(1).astype(np.float32)
    sqh, sqm, sql = _split3(sq)
    sdh, sdm, sdl = _split3(sd)
    oq = np.ones(len(pq), np.float32)
    od = np.ones(len(pdb), np.float32)
    # pair layout: 6 x-terms, 6 y-terms, 3 q-norm rows, 3 d-norm rows
    q_aug = np.stack([
        xqh, xqh, xqm, xqh, xql, xqm,
        yqh, yqh, yqm, yqh, yql, yqm,
        sqh, sqm, sql,
        oq, oq, oq,
    ])
    d_aug = np.stack([
        -2 * xdh, -2 * xdm, -2 * xdh, -2 * xdl, -2 * xdh, -2 * xdm,
        -2 * ydh, -2 * ydm, -2 * ydh, -2 * ydl, -2 * ydh, -2 * ydm,
        od, od, od,
        sdh, sdm, sdl,
    ])
    return (q_aug.astype(ml_dtypes.bfloat16), d_aug.astype(ml_dtypes.bfloat16))


def kernel(points1, points2):
    points1 = np.asarray(points1, dtype=np.float32)
    points2 = np.asarray(points2, dtype=np.float32)
    assert points1.shape == (B, N, 2) and points2.shape == (B, M, 2)

    if "nc" not in _CACHE:
        _CACHE["nc"] = _build_program()
    nc = _CACHE["nc"]

    in_maps = []
    for b in range(B):
        qa_a, db_a = _make_aug(points1[b], points2[b])
        qa_b, db_b = _make_aug(points2[b], points1[b])
        in_maps.append({"qa_a": qa_a, "db_a": db_a, "qa_b": qa_b, "db_b": db_b})

    trace = bool(int(os.environ.get("CHAMFER_TRACE", "0")))
    kw = {}
    if trace:
        import tempfile
        bass_utils.upload_artifacts = lambda d: f"local:{d}"
        kw = dict(trace=True, tmpdir=tempfile.mkdtemp())
    res = bass_utils.run_bass_kernel_spmd(nc, in_maps, core_ids=list(range(B)), **kw)
    if trace:
        _CACHE["last_exec_time_ns"] = res.exec_time_ns
        _CACHE["last_results"] = res

    total = np.float32(0.0)
    for b in range(B):
        sums = res.results[b]["out"]  # [128, 2]
        cost = sums[:, 0].sum(dtype=np.float32) / np.float32(N) \
            + sums[:, 1].sum(dtype=np.float32) / np.float32(M)
        total = np.float32(total + cost)
    return np.array(total, dtype=np.float32)


# revision 22
# speedup vs baseline: 1.1117x; 1.0268x over previous
"""Chamfer distance 2D (B=8, N=M=8192) Trainium2 Bass kernel.

Strategy
--------
Data-parallel over batch: core b handles batch b (its own 8192x8192 chamfer).

Per core, two symmetric passes (p1->p2 and p2->p1). Each pass computes the
full squared-distance matrix via a K=18 bf16 matmul using split arithmetic:

  d[n, m] = |p1[n]|^2 + |p2[m]|^2 - 2 p1[n].p2[m]

Each fp32 scalar is split into 3 bf16 components (hi/mid/lo, ~24 mantissa
bits recovered); products keep the 6 dominant component pairs per
coordinate, plus 3 rows for each squared-norm term against ones. PSUM
accumulates in fp32, so the matmul emits full squared distances directly
(error ~3e-7 * |p|^2, below the fp32 reference's own rounding noise).

Matmuls are packed 4x with tile_position row groups (K=18 <= 32): row group
r computes chunk 4g+r of the same query tile, filling one [128, 2048] PSUM
tile (4 banks) per group. LDWEIGHTS overlaps across row groups.

Row-min reduction is the bottleneck (only DVE can reduce, only DVE/ACT can
read PSUM, 1 elem/cycle/lane each). Tiles are routed two ways to keep both
engines busy:
  - direct: DVE tensor_reduce(min) straight from PSUM
  - offload: ScalarE copies PSUM -> SBUF bf16, then DVE combines leaves with
    2x-mode bf16 tensor_tensor(min) and one final 1x reduce
Per query tile (4 PSUM tiles), either 3 or 4 tiles take the offload path;
the mix is tuned so DVE and ScalarE finish together.

Output: per-core [128, 2] partition-wise sums of row minima (pass A, B).
Host sums partitions, divides by 8192, adds over batches.
"""
import os
import numpy as np
import ml_dtypes

import concourse.bass as bass
import concourse.tile as tile
from concourse import bacc, mybir, bass_utils
from concourse.bass import ts

f32 = mybir.dt.float32
bf16 = mybir.dt.bfloat16
BIG = 3.0e38

B, N, M = 8, 8192, 8192
NT = 64        # query tiles of 128 per pass
NG = 4         # chunk groups per query tile (each = 4 chunks of 512 = [128, 2048])
K = 18
# per-query-tile offload count (number of the 4 groups routed via ACT).
# t=3 (1 direct PSUM reduce + 3 ACT leaves) for most tiles; t=2 for 1 in 12
# shifts a little work from ScalarE back to DVE (measured balance point).
OFFLOAD_T = [2 if (i % 12) == 0 else 3 for i in range(NT)]

_CACHE = {}


def _build_program():
    nc = bacc.Bacc("TRN2", target_bir_lowering=False, debug=False)
    qa_a = nc.dram_tensor("qa_a", [K, N], bf16, kind="ExternalInput")
    db_a = nc.dram_tensor("db_a", [K, M], bf16, kind="ExternalInput")
    qa_b = nc.dram_tensor("qa_b", [K, M], bf16, kind="ExternalInput")
    db_b = nc.dram_tensor("db_b", [K, N], bf16, kind="ExternalInput")
    out = nc.dram_tensor("out", [128, 2], f32, kind="ExternalOutput")

    with tile.TileContext(nc) as tc:
        with (
            tc.tile_pool(name="inp", bufs=1) as inp,
            tc.tile_pool(name="psum", bufs=2, space="PSUM") as psum,
            tc.tile_pool(name="conv", bufs=10) as conv,
            tc.tile_pool(name="tree", bufs=3) as tree,
            tc.tile_pool(name="big", bufs=2) as big,
            tc.tile_pool(name="accs", bufs=1) as accs,
        ):
            reps = {}
            for name, dram in (("qa_a", qa_a), ("db_a", db_a),
                               ("qa_b", qa_b), ("db_b", db_b)):
                t = inp.tile([128, 8192], bf16, tag=f"rep_{name}")
                for r in range(4):
                    nc.sync.dma_start(out=t[32 * r:32 * r + K, :], in_=dram.ap())
                reps[name] = t

            out_sb = accs.tile([128, 2], f32)

            for ipass, (qa_t, db_t) in enumerate(
                ((reps["qa_a"], reps["db_a"]), (reps["qa_b"], reps["db_b"]))
            ):
                minacc = accs.tile([128, NT, NG], f32, tag=f"minacc{ipass}")
                nc.vector.memset(minacc[:], BIG)

                # Software-pipelined emission: each query tile's tree TTs are
                # deferred by TWO tiles so the DVE queue (in-order) only sees
                # ops whose ACT-converted inputs are already resident.
                pending = []    # [(nt, leaves), ...] awaiting tree emission
                big4 = None     # current batch's root buffer
                pending_chain = None  # (nt0, big4) awaiting collapse chain

                def emit_tree(nt, leaves):
                    nonlocal big4
                    j = nt % 4
                    if j == 0:
                        big4 = big.tile([128, 4, 2048], bf16, tag="big4")
                    while len(leaves) > 2:
                        a = leaves.pop(0)
                        b = leaves.pop(0)
                        u = tree.tile([128, 2048], bf16, tag="tr")
                        nc.vector.tensor_tensor(
                            out=u[:], in0=a[:], in1=b[:], op=mybir.AluOpType.min
                        )
                        leaves.append(u)
                    nc.vector.tensor_tensor(
                        out=big4[:, j, :], in0=leaves[0][:], in1=leaves[1][:],
                        op=mybir.AluOpType.min,
                    )
                    return (nt - 3, big4) if j == 3 else None

                def emit_chain(nt0, roots):
                    c1 = big.tile([128, 4, 1024], bf16, tag="c1")
                    nc.vector.tensor_tensor(
                        out=c1[:], in0=roots[:, :, 0:1024], in1=roots[:, :, 1024:2048],
                        op=mybir.AluOpType.min,
                    )
                    c2 = big.tile([128, 4, 512], bf16, tag="c2")
                    nc.vector.tensor_tensor(
                        out=c2[:], in0=c1[:, :, 0:512], in1=c1[:, :, 512:1024],
                        op=mybir.AluOpType.min,
                    )
                    c3 = big.tile([128, 4, 256], bf16, tag="c3")
                    nc.vector.tensor_tensor(
                        out=c3[:], in0=c2[:, :, 0:256], in1=c2[:, :, 256:512],
                        op=mybir.AluOpType.min,
                    )
                    nc.vector.tensor_reduce(
                        out=minacc[:, nt0:nt0 + 4, NG - 1:NG], in_=c3[:],
                        axis=mybir.AxisListType.X, op=mybir.AluOpType.min,
                    )

                for nt in range(NT):
                    # deferred emission (two tiles behind) FIRST so the DVE
                    # queue leads with ready work
                    if len(pending) >= 2:
                        done = emit_tree(*pending.pop(0))
                        if done is not None:
                            pending_chain = done
                    if pending_chain is not None and (nt % 4) == 2:
                        emit_chain(*pending_chain)
                        pending_chain = None
                    t_off = OFFLOAD_T[nt]
                    leaves = []
                    for g in range(NG):
                        if g < t_off:
                            # conv group: two 2-bank tiles so ACT's fills
                            # ping-pong and its convs run back-to-back
                            sa = psum.tile([128, 1024], f32, tag="ps_s", bufs=2)
                            sb = psum.tile([128, 1024], f32, tag="ps_s", bufs=2)
                            for r in range(4):
                                c = 4 * g + r
                                dst = (sa, sb)[r // 2][:, ts(r % 2, 512)]
                                nc.tensor.matmul(
                                    out=dst,
                                    lhsT=qa_t[32 * r:32 * r + K, ts(nt, 128)],
                                    rhs=db_t[32 * r:32 * r + K, ts(c, 512)],
                                    start=True, stop=True,
                                    tile_position=(32 * r, 0),
                                )
                            leaf = conv.tile([128, 2048], bf16, tag="leaf")
                            nc.scalar.copy(out=leaf[:, 0:1024], in_=sa[:])
                            nc.scalar.copy(out=leaf[:, 1024:2048], in_=sb[:])
                            leaves.append(leaf)
                        else:
                            # direct group: own 4-bank tile, DVE reduces PSUM
                            pt = psum.tile([128, 2048], f32, tag="ps_big", bufs=1)
                            for r in range(4):
                                c = 4 * g + r
                                nc.tensor.matmul(
                                    out=pt[:, ts(r, 512)],
                                    lhsT=qa_t[32 * r:32 * r + K, ts(nt, 128)],
                                    rhs=db_t[32 * r:32 * r + K, ts(c, 512)],
                                    start=True, stop=True,
                                    tile_position=(32 * r, 0),
                                )
                            s = g - t_off  # distinct slot per direct group
                            nc.vector.tensor_reduce(
                                out=minacc[:, nt, s:s + 1], in_=pt[:],
                                axis=mybir.AxisListType.X, op=mybir.AluOpType.min,
                            )
                    pending.append((nt, leaves))

                while pending:
                    done = emit_tree(*pending.pop(0))
                    if pending_chain is not None:
                        emit_chain(*pending_chain)
                        pending_chain = None
                    if done is not None:
                        pending_chain = done
                if pending_chain is not None:
                    emit_chain(*pending_chain)
                    pending_chain = None

                # per-tile min over the NG slots, then sum over tiles
                tile_mins = accs.tile([128, NT], f32, tag=f"tm{ipass}")
                nc.vector.tensor_reduce(
                    out=tile_mins[:], in_=minacc[:],
                    axis=mybir.AxisListType.X, op=mybir.AluOpType.min,
                )
                nc.vector.tensor_reduce(
                    out=out_sb[:, ipass:ipass + 1], in_=tile_mins[:],
                    axis=mybir.AxisListType.X, op=mybir.AluOpType.add,
                )

            nc.sync.dma_start(out=out.ap(), in_=out_sb[:])

    nc.compile()
    return nc


def _split3(x):
    h = x.astype(ml_dtypes.bfloat16).astype(np.float32)
    m = (x - h).astype(ml_dtypes.bfloat16).astype(np.float32)
    l = (x - h - m).astype(ml_dtypes.bfloat16).astype(np.float32)
    return h, m, l


def _make_aug(pq, pdb):
    """Query-side aug [18, Nq] and db-side aug [18, Nd] (bf16)."""
    xqh, xqm, xql = _split3(np.ascontiguousarray(pq[:, 0]))
    yqh, yqm, yql = _split3(np.ascontiguousarray(pq[:, 1]))
    xdh, xdm, xdl = _split3(np.ascontiguousarray(pdb[:, 0]))
    ydh, ydm, ydl = _split3(np.ascontiguousarray(pdb[:, 1]))
    sq = (pq.astype(np.float64) ** 2).sum(1).astype(np.float32)
    sd = (pdb.astype(np.float64) ** 2).sum(1).astype(np.float32)
    sqh, sqm, sql = _split3(sq)
    sdh, sdm, sdl = _split3(sd)
    oq = np.ones(len(pq), np.float32)
    od = np.ones(len(pdb), np.float32)
    # pair layout: 6 x-terms, 6 y-terms, 3 q-norm rows, 3 d-norm rows
    q_aug = np.stack([
        xqh, xqh, xqm, xqh, xql, xqm,
        yqh, yqh, yqm, yqh, yql, yqm,
        sqh, sqm, sql,
        oq, oq, oq,
    ])
    d_aug = np.stack([
        -2 * xdh, -2 * xdm, -2 * xdh, -2 * xdl, -2 * xdh, -2 * xdm,
        -2 * ydh, -2 * ydm, -2 * ydh, -2 * ydl, -2 * ydh, -2 * ydm,
        od, od, od,
        sdh, sdm, sdl,
    ])
    return (q_aug.astype(ml_dtypes.bfloat16), d_aug.astype(ml_dtypes.bfloat16))


def kernel(points1, points2):
    points1 = np.asarray(points1, dtype=np.float32)
    points2 = np.asarray(points2, dtype=np.float32)
    assert points1.shape == (B, N, 2) and points2.shape == (B, M, 2)

    if "nc" not in _CACHE:
        _CACHE["nc"] = _build_program()
    nc = _CACHE["nc"]

    in_maps = []
    for b in range(B):
        qa_a, db_a = _make_aug(points1[b], points2[b])
        qa_b, db_b = _make_aug(points2[b], points1[b])
        in_maps.append({"qa_a": qa_a, "db_a": db_a, "qa_b": qa_b, "db_b": db_b})

    trace = bool(int(os.environ.get("CHAMFER_TRACE", "0")))
    kw = {}
    if trace:
        import tempfile
        bass_utils.upload_artifacts = lambda d: f"local:{d}"
        kw = dict(trace=True, tmpdir=tempfile.mkdtemp())
    res = bass_utils.run_bass_kernel_spmd(nc, in_maps, core_ids=list(range(B)), **kw)
    if trace:
        _CACHE["last_exec_time_ns"] = res.exec_time_ns
        _CACHE["last_results"] = res

    total = np.float32(0.0)
    for b in range(B):
        sums = res.results[b]["out"]  # [128, 2]
        cost = sums[:, 0].sum(dtype=np.float32) / np.float32(N) \
            + sums[:, 1].sum(dtype=np.float32) / np.float32(M)
        total = np.float32(total + cost)
    return np.array(total, dtype=np.float32)
